# revision 32
# baseline (speedup 1.0000x reference)
"""Causal GQA attention (B=2, H=32, KVH=8, N=2048, D=128) on 8 trn2 cores.

Sharding: 64 (batch, q-head) problems; core c gets q-heads [4c, 4c+4) for both
batches (8 independent attention problems per core).  GQA repeat is
`(r kvh)` ordering, so q-head h uses kv-head h % 8 — each per-core q-head is
paired 1:1 with the kv head it needs; no cross-core communication.

Per-core kernel layout ("S-transposed" flash-style, no online softmax needed
since rows are bounded: exp(S*scale) computed without max subtraction):
  - host ships Q^T, K^T as [d=128, n=2048] fp16 tiles (d on partitions),
    V as [j%128 partitions, jblock, d] fp16 augmented with a ones column.
  - S^T[j,i] blocks [128, 512] = matmul(lhsT=K^T block, rhs=Q^T group) in PSUM
  - causal mask on diagonal blocks via an extra one-hot matmul
    (lhsT=I, rhs=lower-triangle of -60000) accumulated into PSUM
  - exp via ScalarE over 3-bank-wide PSUM chunks -> P^T fp16 in SBUF
  - PV: matmul(lhsT=P^T 128-col chunk, rhs=[V | 1]) accumulated over j blocks;
    output column 128 is the softmax denominator (rowsum)
  - finalize: reciprocal of rowsum, per-partition scale, DMA out fp32
"""

import sys

sys.path.insert(0, "/opt/trn_rl_repo")

import numpy as np

import concourse.bass as bass
import concourse.mybir as mybir
from concourse import bacc
import concourse.tile as tile
from concourse.bass_utils import run_bass_kernel_spmd

P = 128
NSEQ = 2048
D = 128
NH = 8          # (batch, q-head) problems per core
NG = 4          # query groups per head
GI = 512        # query rows per group
NJB = 16        # 128-wide key blocks per head
SCALE = 1.0 / np.sqrt(128.0)
MASK_NEG = -60000.0

F16 = mybir.dt.float16
F32 = mybir.dt.float32
PRIO_OFF = 250  # make S-production (QK matmuls + exp) beat PV in the scheduler

_NC_CACHE = {}


def build_nc(trace_scopes=False):
    nc = bacc.Bacc("TRN2", target_bir_lowering=False, debug=False, num_devices=8)

    # per-head packed input: [qT (2048) | kT (2048) | vaug (16*129)] per partition
    W_IN = 2 * NSEQ + NJB * (D + 1)
    inp_d = nc.dram_tensor("inp", [NH, P, W_IN], F16, kind="ExternalInput").ap()
    consts_d = nc.dram_tensor("consts", [P, 2 * P], F16, kind="ExternalInput").ap()
    o_d = nc.dram_tensor("o", [NH, NSEQ, D], F32, kind="ExternalOutput").ap()

    with tile.TileContext(nc) as tc:
        with (
            tc.tile_pool(name="cst", bufs=1) as cpool,
            tc.tile_pool(name="inp", bufs=3) as inpool,
            tc.tile_pool(name="pt", bufs=6) as ppool,
            tc.tile_pool(name="fin", bufs=6) as finpool,
            tc.tile_pool(name="spsum", bufs=2, space="PSUM") as spool,
            tc.tile_pool(name="opsum", bufs=1, space="PSUM") as opool,
        ):
            cst = cpool.tile([P, 2 * P], F16)
            nc.sync.dma_start(cst[:], consts_d)
            # tri01: [128, 128], 0 where p > i else 1 — multiplicative causal
            # mask applied to P^T on the Vector engine (keeps mask matmuls off
            # the PE)
            tri01 = cst[:, 0:P]

            for h in range(NH):
                hin = inpool.tile([P, W_IN], F16, tag="hin")
                if h == 0:
                    # split so group-0's slices (qT[:512], kT[:512], first 4
                    # va blocks) land first and the pipeline starts early
                    cuts = [0, GI, NSEQ, NSEQ + GI, 2 * NSEQ, 2 * NSEQ + 4 * (D + 1), W_IN]
                    order = [(0, 1), (2, 3), (4, 5), (1, 2), (3, 4), (5, 6)]
                    with tc.high_priority(offset=None):
                        for a, b in order[:3]:
                            nc.sync.dma_start(
                                hin[:, cuts[a] : cuts[b]], inp_d[h, :, cuts[a] : cuts[b]]
                            )
                    for a, b in order[3:]:
                        nc.sync.dma_start(
                            hin[:, cuts[a] : cuts[b]], inp_d[h, :, cuts[a] : cuts[b]]
                        )
                else:
                    nc.sync.dma_start(hin[:], inp_d[h])
                qT = hin[:, 0:NSEQ]
                kT = hin[:, NSEQ : 2 * NSEQ]
                va = hin[:, 2 * NSEQ :].rearrange("p (a b) -> p a b", b=D + 1)

                for g in range(NG):
                    O0 = opool.tile([P, 2, D + 1], F32, tag="O0")
                    O1 = opool.tile([P, 2, D + 1], F32, tag="O1")
                    otiles = [(O0, 0), (O0, 1), (O1, 0), (O1, 1)]

                    def pv(Pf, off, jb, ic, g=g, otiles=otiles):
                        ot, sub = otiles[ic]
                        # O0/O1 each hold two 129-wide subtiles in one PSUM
                        # bank: one start (first write) and one stop (last
                        # write) per bank
                        nc.tensor.matmul(
                            ot[:, sub, :],
                            Pf[:, off : off + P],
                            va[:, jb, :],
                            start=(jb == 0 and ic % 2 == 0),
                            stop=(jb == 4 * g + ic and ic % 2 == 1),
                        )

                    # dense key blocks (jb < 4g), 3 per PSUM tile
                    for c0 in range(0, 4 * g, 3):
                        chunk = list(range(c0, min(c0 + 3, 4 * g)))
                        ln = len(chunk)
                        with tc.high_priority(offset=PRIO_OFF):
                            S = spool.tile([P, 3, GI], F32, tag="S")
                            for s, jb in enumerate(chunk):
                                nc.tensor.matmul(
                                    S[:, s, :],
                                    kT[:, jb * P : (jb + 1) * P],
                                    qT[:, g * GI : (g + 1) * GI],
                                    start=True,
                                    stop=True,
                                )
                            Pt = ppool.tile([P, 3, GI], F16, tag="P")
                            nc.scalar.activation(
                                Pt[:, 0:ln, :],
                                S[:, 0:ln, :],
                                mybir.ActivationFunctionType.Exp,
                                scale=float(SCALE),
                            )
                        Pf = Pt[:].rearrange("p a b -> p (a b)")
                        for s, jb in enumerate(chunk):
                            for ic in range(4):
                                pv(Pf, s * GI + ic * P, jb, ic)

                    # diagonal group (jb = 4g+r, r=0..3): only the unmasked
                    # suffix of each block is computed, packed contiguously:
                    #   bank0: r0 (512) | bank1: r1 (384) + r3 (128) |
                    #   bank2: r2 (256)  -> one 1280-wide exp
                    with tc.high_priority(offset=PRIO_OFF):
                        S = spool.tile([P, 3, GI], F32, tag="S")
                        Sf = S[:].rearrange("p a b -> p (a b)")
                        roff = {0: 0, 1: GI, 3: GI + 384, 2: 2 * GI}
                        rw = {0: 512, 1: 384, 3: 128, 2: 256}
                        # per bank: one accumulation group (one start, one stop)
                        bank_rs = [(0,), (1, 3), (2,)]
                        for rs in bank_rs:
                            for pos, r in enumerate(rs):
                                jb = 4 * g + r
                                nc.tensor.matmul(
                                    Sf[:, roff[r] : roff[r] + rw[r]],
                                    kT[:, jb * P : (jb + 1) * P],
                                    qT[:, g * GI + r * P : (g + 1) * GI],
                                    start=(pos == 0),
                                    stop=(pos == len(rs) - 1),
                                )
                        Pt = ppool.tile([P, 3, GI], F16, tag="P")
                        Pf = Pt[:].rearrange("p a b -> p (a b)")
                        if h == 0 and g == 0:
                            # prime the pipeline: first exp only needs bank 0
                            nc.scalar.activation(
                                Pf[:, 0:512],
                                Sf[:, 0:512],
                                mybir.ActivationFunctionType.Exp,
                                scale=float(SCALE),
                            )
                            nc.scalar.activation(
                                Pf[:, 512:1280],
                                Sf[:, 512:1280],
                                mybir.ActivationFunctionType.Exp,
                                scale=float(SCALE),
                            )
                        else:
                            nc.scalar.activation(
                                Pf[:, 0:1280],
                                Sf[:, 0:1280],
                                mybir.ActivationFunctionType.Exp,
                                scale=float(SCALE),
                            )
                        # causal triangle: zero masked P^T entries on DVE
                        for r in range(4):
                            nc.vector.tensor_tensor(
                                Pf[:, roff[r] : roff[r] + P],
                                Pf[:, roff[r] : roff[r] + P],
                                tri01,
                                mybir.AluOpType.mult,
                            )
                    for r in range(4):
                        for ic in range(r, 4):
                            pv(Pf, roff[r] + (ic - r) * P, 4 * g + r, ic)

                    osb = finpool.tile([P, 4, D], F32, tag="osb")
                    for ic in range(4):
                        ot, sub = otiles[ic]
                        rec = finpool.tile([P, 1], F32, tag="rec")
                        nc.vector.reciprocal(rec[:], ot[:, sub, D : D + 1])
                        nc.vector.tensor_scalar_mul(
                            osb[:, ic, :], ot[:, sub, 0:D], rec[:]
                        )
                    nc.sync.dma_start(
                        o_d[h, g * GI : (g + 1) * GI, :].rearrange(
                            "(a p) d -> p a d", p=P
                        ),
                        osb[:],
                    )
    nc.compile()
    return nc


def _get_nc():
    if "nc" not in _NC_CACHE:
        _NC_CACHE["nc"] = build_nc()
    return _NC_CACHE["nc"]


def make_consts():
    pp = np.arange(P)[:, None]
    ii = np.arange(P)[None, :]
    tri01 = np.where(pp > ii, np.float16(0.0), np.float16(1.0)).astype(np.float16)
    return np.concatenate([tri01, np.zeros((P, P), np.float16)], axis=1)


def make_in_maps(q, k, v):
    """Shard full inputs into 8 per-core input maps (host-side layout prep)."""
    consts = make_consts()
    W_IN = 2 * NSEQ + NJB * (D + 1)
    in_maps = []
    for c in range(8):
        inp = np.empty((NH, P, W_IN), dtype=np.float16)
        i = 0
        for b in range(2):
            for qh in range(4 * c, 4 * c + 4):
                kvh = qh % 8
                inp[i, :, 0:NSEQ] = q[b, qh].T
                inp[i, :, NSEQ : 2 * NSEQ] = k[b, kvh].T
                va = inp[i, :, 2 * NSEQ :].reshape(P, NJB, D + 1)
                # v[b,kvh]: [2048, 128] -> [jb, p, d] -> [p, jb, d]
                va[:, :, :D] = v[b, kvh].reshape(NJB, P, D).transpose(1, 0, 2)
                va[:, :, D] = 1.0
                i += 1
        in_maps.append({"inp": inp, "consts": consts})
    return in_maps


def assemble_output(results):
    out = np.empty((2, 32, NSEQ, D), dtype=np.float32)
    for c in range(8):
        o = results[c]["o"]
        i = 0
        for b in range(2):
            for qh in range(4 * c, 4 * c + 4):
                out[b, qh] = o[i]
                i += 1
    return out


def _install_ntff_hook():
    """The agent image's antenv lacks axon_hooks; inject a shim so
    run_bass_kernel_spmd(trace=True) can reach the NTFF profiler in
    libaxon_pjrt.so. Only needed for profiling runs."""
    import types

    if "antenv.axon_hooks" in sys.modules:
        return
    mod = types.ModuleType("antenv.axon_hooks")
    _h = [None]
    mod.set_axon_ntff_profile_hook = lambda h: _h.__setitem__(0, h)
    mod.get_axon_ntff_profile_hook = lambda: _h[0]
    sys.modules["antenv.axon_hooks"] = mod
    import antenv

    antenv.axon_hooks = mod
    if "/root/.axon_site" not in sys.path:
        sys.path.insert(0, "/root/.axon_site")
    from trn_agent_boot.trn_boot import _ntff_profile_via_ctypes

    hook = _ntff_profile_via_ctypes("/opt/axon/libaxon_pjrt.so")
    if hook is not None:
        mod.set_axon_ntff_profile_hook(hook)

    # avoid S3-ish artifact upload in this container
    import concourse.bass_utils as bu

    bu.upload_artifacts = lambda tmpdir: tmpdir


def kernel(q, k, v, _trace=False, _trace_kwargs=None):
    q = np.asarray(q, dtype=np.float32)
    k = np.asarray(k, dtype=np.float32)
    v = np.asarray(v, dtype=np.float32)
    assert q.shape == (2, 32, NSEQ, D), q.shape
    assert k.shape == (2, 8, NSEQ, D), k.shape
    assert v.shape == (2, 8, NSEQ, D), v.shape

    nc = _get_nc()
    in_maps = make_in_maps(q, k, v)
    kwargs = {}
    if _trace:
        _install_ntff_hook()
        kwargs["trace"] = True
        kwargs.update(_trace_kwargs or {})
    res = run_bass_kernel_spmd(nc, in_maps, core_ids=list(range(8)), **kwargs)
    out = assemble_output(res.results)
    if _trace:
        return out, res
    return out
